# revision 3
# baseline (speedup 1.0000x reference)
"""DemandMap (histogram_binning) Trainium2 Bass kernel.

Problem (hardcoded from the reference):
  W = H = 2048 site grid, NBX = NBY = 2048 bins -> binW = binH = 1.0.
  Sites sit at integer (r, c) with r = idx // H, c = idx % H; all site
  types have sx = 1.0, so each site contributes ONLY to bin row i = r.
  Along c the footprint covers a short window:
    type 1 (sy=1.0):  cap1[r,c] = m1[r,c]
    type 2 (sy=2.5):  cap2[r,c] = m2[r,c] + m2[r,c-1] + 0.5*m2[r,c-2]
    type 3 (sy=5.0):  cap3[r,c] = sum_{k=0..4} m3[r,c-k]
  (mt = site_type_map == t, contributions with c-k < 0 drop out, and
  bins beyond NBY-1 simply don't exist -- no clamping terms survive.)
  Output tuple: (1-cap1, 1-cap1, 1-cap2, 1-cap3), binArea = 1.0.

Sharding: rows r are split evenly over the 8 cores (perfectly local:
no halo along r because sx=1, so no all-reduce is needed at all).

Per core: input slab [256, 2048] int32, three output slabs [256, 2048].
Each core runs 2 SBUF tiles of [128 partitions, 2048 free].  The column
window sums become shifted adds along the free axis; left borders of the
shifted buffers are zeroed once so out-of-range taps vanish.

DVE schedule per tile (A = AluOpType), everything fused so the "+1"
(binArea) rides in the scalar slots -- 8 full-tile ops total:
  o0 = (x != 1)                                   # = 1 - cap1
  p2 = -(x == 2)           (is_equal, mult -1)
  t2 = (s1(p2) + 1) + p2                          # 1 - m2 - s1(m2)
  o2 = (s2(p2) * 0.5) + t2                        # = 1 - cap2
  p3 = -(x == 3)
  a3 = p3 + s1(p3)
  b3 = (s2(a3) + 1) + a3                          # 1 - 4-tap sum
  o3 = s4(p3) + b3                                # = 1 - cap3
where sk(v) reads v shifted k columns toward +c (tap at c-k).

Outputs are stored as bf16: every reachable value lies in
{1, 0.5, 0, -0.5, ..., -4}, all exactly representable in bf16, so the
host-side cast back to f32 is exact.
"""

from contextlib import ExitStack

import numpy as np
import ml_dtypes  # noqa: F401  (bf16 numpy dtype registration)

import concourse.bass as bass
import concourse.mybir as mybir
from concourse.bass_utils import run_bass_kernel_spmd

N_CORES = 8
W = 2048          # rows r (site x / bin x)
C = 2048          # cols c (site y / bin y)
R_PER = W // N_CORES   # 256 rows per core
P = 128                # SBUF partitions
NT = R_PER // P        # tiles per core

_A = mybir.AluOpType

# toggles for perf experiments
CDT = mybir.dt.float32    # compute dtype for intermediates
ODT = mybir.dt.bfloat16   # output dtype (exact for all reachable values)

LAST_RESULTS = None  # BassKernelResults of the most recent run (for test.py)


def _build_program():
    nc = bass.Bass()
    stm = nc.dram_tensor("stm", [R_PER, C], mybir.dt.int32, kind="ExternalInput")
    o0d = nc.dram_tensor("o0", [R_PER, C], ODT, kind="ExternalOutput")
    o2d = nc.dram_tensor("o2", [R_PER, C], ODT, kind="ExternalOutput")
    o3d = nc.dram_tensor("o3", [R_PER, C], ODT, kind="ExternalOutput")

    with ExitStack() as ctx:
        def sb(nm, shape, dt):
            return [
                ctx.enter_context(nc.sbuf_tensor(f"{nm}{i}", shape, dt))
                for i in range(NT)
            ]

        xt = sb("xt", [P, C], mybir.dt.int32)
        p2b = sb("p2b", [P, C + 2], CDT)
        p3b = sb("p3b", [P, C + 4], CDT)
        a3b = sb("a3b", [P, C + 2], CDT)
        t2 = sb("t2", [P, C], CDT)
        b3 = sb("b3", [P, C], CDT)
        o0 = sb("o0s", [P, C], ODT)
        o2 = sb("o2s", [P, C], ODT)
        o3 = sb("o3s", [P, C], ODT)

        in_sems = [
            ctx.enter_context(nc.semaphore(f"in_sem{i}")) for i in range(NT)
        ]
        cmp_sem = ctx.enter_context(nc.semaphore())
        out_sem = ctx.enter_context(nc.semaphore())
        block = ctx.enter_context(nc.Block())

        @block.sync
        def _(sync):
            for i in range(NT):
                sync.dma_start(
                    out=xt[i][:], in_=stm[i * P : (i + 1) * P, :]
                ).then_inc(in_sems[i], 16)
            for i in range(NT):
                sync.wait_ge(cmp_sem, i + 1)
                rows = slice(i * P, (i + 1) * P)
                sync.dma_start(out=o0d[rows, :], in_=o0[i][:]).then_inc(out_sem, 16)
                sync.dma_start(out=o2d[rows, :], in_=o2[i][:]).then_inc(out_sem, 16)
                sync.dma_start(out=o3d[rows, :], in_=o3[i][:]).then_inc(out_sem, 16)
            sync.wait_ge(out_sem, NT * 3 * 16)

        @block.vector
        def _(vector):
            for i in range(NT):
                vector.memset(p2b[i][:, 0:2], 0.0)
                vector.memset(p3b[i][:, 0:4], 0.0)
                vector.memset(a3b[i][:, 0:2], 0.0)
            for i in range(NT):
                vector.wait_ge(in_sems[i], 16)
                x = xt[i][:]
                p2, p3, a3 = p2b[i], p3b[i], a3b[i]
                vector.tensor_scalar(o0[i][:], x, 1, None, _A.not_equal)
                vector.tensor_scalar(p2[:, 2 : C + 2], x, 2, -1.0, _A.is_equal, _A.mult)
                vector.scalar_tensor_tensor(
                    t2[i][:], p2[:, 1 : C + 1], 1.0, p2[:, 2 : C + 2], _A.add, _A.add
                )
                vector.scalar_tensor_tensor(
                    o2[i][:], p2[:, 0:C], 0.5, t2[i][:], _A.mult, _A.add
                )
                vector.tensor_scalar(p3[:, 4 : C + 4], x, 3, -1.0, _A.is_equal, _A.mult)
                vector.tensor_tensor(
                    a3[:, 2 : C + 2], p3[:, 4 : C + 4], p3[:, 3 : C + 3], _A.add
                )
                vector.scalar_tensor_tensor(
                    b3[i][:], a3[:, 0:C], 1.0, a3[:, 2 : C + 2], _A.add, _A.add
                )
                vector.tensor_tensor(
                    o3[i][:], p3[:, 0:C], b3[i][:], _A.add
                ).then_inc(cmp_sem, 1)

    return nc


def kernel(site_type_map, node_size_x, node_size_y, width, height,
           num_bins_x, num_bins_y, xl, xh, yl, yh):
    global LAST_RESULTS
    stm = np.ascontiguousarray(np.asarray(site_type_map, dtype=np.int32)).reshape(W, C)

    nc = _build_program()
    in_maps = [
        {"stm": np.ascontiguousarray(stm[k * R_PER : (k + 1) * R_PER, :])}
        for k in range(N_CORES)
    ]
    res = run_bass_kernel_spmd(nc, in_maps, core_ids=list(range(N_CORES)))
    LAST_RESULTS = res

    def gather(name):
        full = np.concatenate(
            [np.asarray(res.results[k][name]) for k in range(N_CORES)], axis=0
        )
        return full.astype(np.float32)

    out0 = gather("o0")
    out2 = gather("o2")
    out3 = gather("o3")
    return (out0, out0, out2, out3)


# revision 4
# speedup vs baseline: 1.7451x; 1.7451x over previous
"""DemandMap (histogram_binning) Trainium2 Bass kernel.

Problem (hardcoded from the reference):
  W = H = 2048 site grid, NBX = NBY = 2048 bins -> binW = binH = 1.0.
  Sites sit at integer (r, c) with r = idx // H, c = idx % H; all site
  types have sx = 1.0, so each site contributes ONLY to bin row i = r.
  Along c the footprint covers a short window:
    type 1 (sy=1.0):  cap1[r,c] = m1[r,c]
    type 2 (sy=2.5):  cap2[r,c] = m2[r,c] + m2[r,c-1] + 0.5*m2[r,c-2]
    type 3 (sy=5.0):  cap3[r,c] = sum_{k=0..4} m3[r,c-k]
  (mt = site_type_map == t; taps with c-k < 0 drop out; bins beyond
  NBY-1 don't exist, so no clamping terms survive.)
  Output tuple: (1-cap1, 1-cap1, 1-cap2, 1-cap3), binArea = 1.0.

Sharding: rows r split evenly over 8 cores — perfectly local (sx=1
means no halo along r), so no collectives at all.

Per core: slab [256, 2048] as 2 SBUF tiles of [128 part, 2048 free].
The site-type map is shipped as bf16 (values 0..3, exact) so the DVE
compare ops hit the 4x perf mode.  Column-window sums become shifted
adds along the free axis; shifted buffers carry zeroed left borders so
out-of-range taps vanish.  All intermediates and outputs are bf16 —
every reachable value (integers/halves in [-4, 1]) is exact in bf16,
so the host-side cast back to f32 is exact.

Engine split per tile (p2 = -(x==2), p3 = -(x==3); sk = read shifted k
columns, i.e. tap at c-k):
  DVE : p2, p3, o0=(x!=1), a2=p2+s1(p2), a3=p3+s1(p3), b3=a3+s2(a3),
        o2=a2+h2, o3=b3+g3
  ACT : h2 = 0.5*s2(p2)+1,  g3 = s4(p3)+1      (Copy activation)
  POOL: zero the borders once at start
  SP  : all DMA (loads, then per-output stores as results finish)
"""

from contextlib import ExitStack

import numpy as np
import ml_dtypes

import concourse.bass as bass
import concourse.mybir as mybir
from concourse.bass_utils import run_bass_kernel_spmd

N_CORES = 8
W = 2048               # rows r (site x / bin x)
C = 2048               # cols c (site y / bin y)
R_PER = W // N_CORES   # 256 rows per core
P = 128                # SBUF partitions
NT = R_PER // P        # tiles per core

_A = mybir.AluOpType
BF = mybir.dt.bfloat16

LAST_RESULTS = None  # BassKernelResults of the most recent run (for test.py)


def _build_program():
    nc = bass.Bass()
    stm = nc.dram_tensor("stm", [R_PER, C], BF, kind="ExternalInput")
    o0d = nc.dram_tensor("o0", [R_PER, C], BF, kind="ExternalOutput")
    o2d = nc.dram_tensor("o2", [R_PER, C], BF, kind="ExternalOutput")
    o3d = nc.dram_tensor("o3", [R_PER, C], BF, kind="ExternalOutput")

    with ExitStack() as ctx:
        def sb(nm, cols):
            return [
                ctx.enter_context(nc.sbuf_tensor(f"{nm}{i}", [P, cols], BF))
                for i in range(NT)
            ]

        xt = sb("xt", C)
        p2b = sb("p2b", C + 4)   # data at cols 4..C+4, zero border cols 2..3
        p3b = sb("p3b", C + 8)   # data at cols 8..C+8, zero border cols 4..7
        a3b = sb("a3b", C + 4)   # data at cols 4..C+4, zero border cols 2..3
        a2 = sb("a2", C)
        b3 = sb("b3", C)
        h2 = sb("h2", C)
        g3 = sb("g3", C)
        o0 = sb("o0s", C)
        o2 = sb("o2s", C)
        o3 = sb("o3s", C)

        in_sems = [ctx.enter_context(nc.semaphore(f"in_sem{i}")) for i in range(NT)]
        sem_ms = ctx.enter_context(nc.semaphore("sem_ms"))    # borders done
        sem_p2 = ctx.enter_context(nc.semaphore("sem_p2"))    # p2 ready (per tile)
        sem_p3 = ctx.enter_context(nc.semaphore("sem_p3"))    # p3 ready (per tile)
        sem_h = ctx.enter_context(nc.semaphore("sem_h"))      # h2 ready
        sem_g = ctx.enter_context(nc.semaphore("sem_g"))      # g3 ready
        sem_st = ctx.enter_context(nc.semaphore("sem_st"))    # outputs ready (3/tile)
        out_sem = ctx.enter_context(nc.semaphore("out_sem"))
        block = ctx.enter_context(nc.Block())

        @block.sync
        def _(sync):
            for i in range(NT):
                sync.dma_start(
                    out=xt[i][:], in_=stm[i * P : (i + 1) * P, :]
                ).then_inc(in_sems[i], 16)
            for i in range(NT):
                rows = slice(i * P, (i + 1) * P)
                sync.wait_ge(sem_st, 3 * i + 1)
                sync.dma_start(out=o0d[rows, :], in_=o0[i][:]).then_inc(out_sem, 16)
                sync.wait_ge(sem_st, 3 * i + 2)
                sync.dma_start(out=o2d[rows, :], in_=o2[i][:]).then_inc(out_sem, 16)
                sync.wait_ge(sem_st, 3 * i + 3)
                sync.dma_start(out=o3d[rows, :], in_=o3[i][:]).then_inc(out_sem, 16)
            sync.wait_ge(out_sem, NT * 3 * 16)

        @block.gpsimd
        def _(gp):
            for i in range(NT):
                gp.memset(p2b[i][:, 2:4], 0.0)
                gp.memset(p3b[i][:, 4:8], 0.0)
                gp.memset(a3b[i][:, 2:4], 0.0)
            gp.sem_inc(sem_ms, 1)

        @block.scalar
        def _(act):
            Copy = mybir.ActivationFunctionType.Copy
            for i in range(NT):
                act.wait_ge(sem_p2, i + 1)
                act.activation(h2[i][:], p2b[i][:, 2 : C + 2], Copy,
                               bias=1.0, scale=0.5).then_inc(sem_h, 1)
                act.wait_ge(sem_p3, i + 1)
                act.activation(g3[i][:], p3b[i][:, 4 : C + 4], Copy,
                               bias=1.0, scale=1.0).then_inc(sem_g, 1)

        @block.vector
        def _(v):
            v.wait_ge(sem_ms, 1)
            for i in range(NT):
                v.wait_ge(in_sems[i], 16)
                x = xt[i][:]
                p2, p3, a3 = p2b[i], p3b[i], a3b[i]
                v.tensor_scalar(p2[:, 4 : C + 4], x, 2, -1.0,
                                _A.is_equal, _A.mult).then_inc(sem_p2, 1)
                v.tensor_scalar(p3[:, 8 : C + 8], x, 3, -1.0,
                                _A.is_equal, _A.mult).then_inc(sem_p3, 1)
                v.tensor_scalar(o0[i][:], x, 1, None,
                                _A.not_equal).then_inc(sem_st, 1)
                v.tensor_tensor(a2[i][:], p2[:, 4 : C + 4], p2[:, 3 : C + 3], _A.add)
                v.tensor_tensor(a3[:, 4 : C + 4], p3[:, 8 : C + 8],
                                p3[:, 7 : C + 7], _A.add)
                v.tensor_tensor(b3[i][:], a3[:, 4 : C + 4], a3[:, 2 : C + 2], _A.add)
                v.wait_ge(sem_h, i + 1)
                v.tensor_tensor(o2[i][:], a2[i][:], h2[i][:],
                                _A.add).then_inc(sem_st, 1)
                v.wait_ge(sem_g, i + 1)
                v.tensor_tensor(o3[i][:], b3[i][:], g3[i][:],
                                _A.add).then_inc(sem_st, 1)

    return nc


def kernel(site_type_map, node_size_x, node_size_y, width, height,
           num_bins_x, num_bins_y, xl, xh, yl, yh):
    global LAST_RESULTS
    stm = np.asarray(site_type_map, dtype=np.int32).reshape(W, C)
    stm_bf = stm.astype(ml_dtypes.bfloat16)  # values 0..3: exact in bf16

    nc = _build_program()
    in_maps = [
        {"stm": np.ascontiguousarray(stm_bf[k * R_PER : (k + 1) * R_PER, :])}
        for k in range(N_CORES)
    ]
    res = run_bass_kernel_spmd(nc, in_maps, core_ids=list(range(N_CORES)))
    LAST_RESULTS = res

    def gather(name):
        full = np.concatenate(
            [np.asarray(res.results[k][name]) for k in range(N_CORES)], axis=0
        )
        return full.astype(np.float32)

    out0 = gather("o0")
    out2 = gather("o2")
    out3 = gather("o3")
    return (out0, out0, out2, out3)
